# revision 1
# baseline (speedup 1.0000x reference)
"""MultiHeadAttention (dense, B=4 S=2048 D=1024 H=16) + residual + LayerNorm
on 8 Trainium2 NeuronCores.

Sharding: core c handles batch b=c//2 and head group g=c%2 (8 of 16 heads),
all 2048 query tokens. After attention, the pair (2b, 2b+1) exchanges
normalized per-head context (fp8, AllToAll) so each core does the full-d_in
output projection + residual + LayerNorm for its own 1024 tokens. No
reduction collective is needed.

Perf design:
  - projections / att@V / fc run as fp8 (e4m3) matmuls with
    MatmulPerfMode.DoubleRow (2 k-tiles per instruction, ~1.4-2x).
  - weights are pre-scaled x32 on host so fp8 sees ~unit-variance values.
  - Q^T/K^T are written bf16 pre-scaled by a=sqrt(4/(8*ln2)) so the score
    PSUM value is s' = 4*z/ln2 (z = score/sqrt(dk)) -- i.e. fp8e5m2
    exponent-units x4.  Softmax exp is then either
      scalar engine: Exp(s' * ln2/4) -> fp8e5  (exact)
      vector engine: uint8(max(s' + C2, 0)) bit-viewed as fp8e5 (Schraudolph)
    split across both engines to break the scalar-activation bottleneck.
  - att@V: lhsT = V(fp8 x32, ones-col=2.0 at col 64) DoubleRow over key-chunk
    pairs; pv row 64 = 2*sum(exp) gives softmax denominators for free.
  - normalize: reciprocal_approx_fast per head-pair + SBUF->SBUF broadcast
    DMA + one tensor_tensor mult writing fp8 context (x16) for the fc.
"""

import numpy as np
import ml_dtypes

import concourse.bass as bass
import concourse.mybir as mybir
import concourse.tile as tile
from concourse import bacc
from concourse.bass_utils import run_bass_kernel_spmd

BF16 = mybir.dt.bfloat16
F32 = mybir.dt.float32
FP8 = mybir.dt.float8e4
FP8E5 = mybir.dt.float8e5
U8 = mybir.dt.uint8
AF = mybir.ActivationFunctionType
OP = mybir.AluOpType
DR = mybir.MatmulPerfMode.DoubleRow

B = 4
S = 2048
D = 1024
HL = 8          # heads per core
DK = 64
DH = HL * DK    # 512 local projection width
P = 128
KC = S // P     # 16 key chunks
QC = 2          # query halves of 1024
LN_EPS = 1e-5
LN2 = float(np.log(2.0))
A_QK = float(np.sqrt(4.0 / (8.0 * LN2)))   # per-side Q/K scale
C2 = 59.8                                   # exp trick: t = max(s' + C2, 0)
WS = 32.0                                   # host weight scale for fp8
EXP_SCALE = LN2 / 4.0                       # scalar-engine exp scale

# (kc, hb) tiles owned by the vector engine (rest: scalar engine)
def _vec_owned(kc, hb):
    return ((2 * kc + hb) % 7) < 3

_NC_CACHE = None
_LAST_RES = None


def build_nc(dbg=False):
    nc = bacc.Bacc(
        None, target_bir_lowering=False, num_devices=8, dynamic_dma_scratch_size=2048
    )

    xqT = nc.declare_dram_parameter("xqT", [D, S], FP8, isOutput=False)
    xkT = nc.declare_dram_parameter("xkT", [D, S], FP8, isOutput=False)
    xvT = nc.declare_dram_parameter("xvT", [D, S], FP8, isOutput=False)
    wqT = nc.declare_dram_parameter("wqT", [D, DH], FP8, isOutput=False)
    wkT = nc.declare_dram_parameter("wkT", [D, DH], FP8, isOutput=False)
    wvT = nc.declare_dram_parameter("wvT", [D, DH], FP8, isOutput=False)
    woT = nc.declare_dram_parameter("woT", [DH, D], FP8, isOutput=False)
    bq_d = nc.declare_dram_parameter("bq", [DH], F32, isOutput=False)   # *A_QK
    bk_d = nc.declare_dram_parameter("bk", [DH], F32, isOutput=False)   # *A_QK
    bv_d = nc.declare_dram_parameter("bv", [1, DH], F32, isOutput=False)  # *32
    gam_d = nc.declare_dram_parameter("gamma", [1, D], F32, isOutput=False)
    bet_d = nc.declare_dram_parameter("beta", [1, D], F32, isOutput=False)
    qrb_d = nc.declare_dram_parameter("qrb", [S // 2, D], F32, isOutput=False)
    out_d = nc.declare_dram_parameter("out", [S // 2, D], F32, isOutput=True)

    if dbg:
        dbg_qt = nc.declare_dram_parameter("dbg_qt", [P, 4, S], BF16, isOutput=True)
        dbg_kt = nc.declare_dram_parameter("dbg_kt", [P, 4, S], BF16, isOutput=True)
        dbg_vo = nc.declare_dram_parameter("dbg_vo", [P, 8, 2, HL, 66], FP8, isOutput=True)
        dbg_exp = nc.declare_dram_parameter("dbg_exp", [P, KC, 2048], FP8E5, isOutput=True)
        dbg_att = nc.declare_dram_parameter("dbg_att", [P, 4, S], FP8, isOutput=True)
        dbg_rec = nc.declare_dram_parameter("dbg_rec", [HL * QC, 1024], F32, isOutput=True)
        dbg_fc = nc.declare_dram_parameter("dbg_fc", [512, D], BF16, isOutput=True)

    rec_dram = nc.dram_tensor("rec_dram", [HL * QC, 1024], F32)
    sums_dram = nc.dram_tensor("sums_dram", [HL * QC, 1024], BF16)
    # ReduceScatter bounce buffers, 2 chunks per q-half (bf16 partials).
    # chunk rows = [256 tokens of pair-member 0, 256 tokens of member 1] so the
    # scatter hands each core its own tokens.
    cc_in = [nc.dram_tensor(f"cc_in{i}", [512, D], BF16) for i in range(2 * QC)]
    cc_out = [nc.dram_tensor(f"cc_out{i}", [256, D], BF16) for i in range(2 * QC)]
    groups = [[0, 1], [2, 3], [4, 5], [6, 7]]

    with tile.TileContext(nc) as tc:
        with (
            tc.tile_pool(name="pers", bufs=1) as pers,
            tc.tile_pool(name="ps", bufs=2, space="PSUM") as ps,
        ):
            QT = pers.tile([P, 4, S], BF16, tag="QT")
            KT = pers.tile([P, 4, S], BF16, tag="KT")
            # V fp8 x32, DoubleRow pair layout [p, kpair, ktile, head, 66]
            VO = pers.tile([P, 8, 2, HL, 66], FP8, tag="VO")
            # normalized context (x16), fp8, d_local = chunk*128 + p
            ATT = pers.tile([P, 4, S], FP8, tag="ATT")
            WOp = pers.tile([P, 4, D], FP8, tag="WOp")
            BQK = pers.tile([P, 8], F32, tag="BQK")  # cols 0-3 bq*a, 4-7 bk*a

            nc.sync.dma_start(out=WOp, in_=woT.ap().rearrange("(c p) d -> p c d", p=P))
            nc.sync.dma_start(out=BQK[:, 0:4], in_=bq_d.ap().rearrange("(c p) -> p c", p=P))
            nc.sync.dma_start(out=BQK[:, 4:8], in_=bk_d.ap().rearrange("(c p) -> p c", p=P))
            nc.gpsimd.memset(VO[:, :, :, :, 64:65], 2.0)

            # ---------------- projections (fp8 DoubleRow) ----------------
            with tc.tile_pool(name="inp", bufs=1) as inp:
                XQ = inp.tile([P, 8, S], FP8, tag="XQ")
                XK = inp.tile([P, 8, S], FP8, tag="XK")
                XV = inp.tile([P, 8, S], FP8, tag="XV")
                WQ = inp.tile([P, 8, DH], FP8, tag="WQ")
                WK = inp.tile([P, 8, DH], FP8, tag="WK")
                WV = inp.tile([P, 8, DH], FP8, tag="WV")
                BVB = inp.tile([P, DH], F32, tag="BVB")

                nc.sync.dma_start(out=WV, in_=wvT.ap().rearrange("(c p) n -> p c n", p=P))
                nc.sync.dma_start(out=BVB, in_=bv_d.ap().to_broadcast([P, DH]))
                # chunk XV by token quarters so V-proj starts early
                for tq in range(4):
                    nc.sync.dma_start(
                        out=XV[:, :, tq * 512 : (tq + 1) * 512],
                        in_=xvT.ap().rearrange("(c p) s -> p c s", p=P)[
                            :, :, tq * 512 : (tq + 1) * 512
                        ],
                    )
                nc.sync.dma_start(out=WQ, in_=wqT.ap().rearrange("(c p) n -> p c n", p=P))
                nc.sync.dma_start(out=WK, in_=wkT.ap().rearrange("(c p) n -> p c n", p=P))
                for tq in range(2):
                    nc.sync.dma_start(
                        out=XQ[:, :, tq * 1024 : (tq + 1) * 1024],
                        in_=xqT.ap().rearrange("(c p) s -> p c s", p=P)[
                            :, :, tq * 1024 : (tq + 1) * 1024
                        ],
                    )
                for tq in range(2):
                    nc.sync.dma_start(
                        out=XK[:, :, tq * 1024 : (tq + 1) * 1024],
                        in_=xkT.ap().rearrange("(c p) s -> p c s", p=P)[
                            :, :, tq * 1024 : (tq + 1) * 1024
                        ],
                    )

                # V = (v @ Wv.T)*32 + 32*bv, sliced per head into VO (fp8)
                for tokc in range(KC):
                    psv = ps.tile([P, 1024], F32, tag="sc")
                    for kp in range(4):
                        nc.tensor.matmul(
                            psv[:, 0:DH],
                            lhsT=XV[:, 2 * kp : 2 * kp + 2, tokc * P : (tokc + 1) * P],
                            rhs=WV[:, 2 * kp : 2 * kp + 2, :],
                            start=(kp == 0),
                            stop=(kp == 3),
                            perf_mode=DR,
                        )
                    nc.vector.tensor_tensor(
                        VO[:, tokc // 2, tokc % 2, :, 0:DK],
                        psv[:, 0:DH].rearrange("p (h d) -> p h d", h=HL),
                        BVB.rearrange("p (h d) -> p h d", h=HL),
                        OP.add,
                    )

                # Q^T / K^T = a*(W @ x^T + b)  (d_out on partitions), bf16
                for mc in range(4):
                    for which, WX, XX, outT, bcol in (
                        (0, WQ, XQ, QT, 0),
                        (1, WK, XK, KT, 4),
                    ):
                        for nt in range(2):
                            psq = ps.tile([P, 1024], F32, tag="sc")
                            for kp in range(4):
                                for half in range(2):
                                    nc.tensor.matmul(
                                        psq[:, half * 512 : (half + 1) * 512],
                                        lhsT=WX[:, 2 * kp : 2 * kp + 2, mc * P : (mc + 1) * P],
                                        rhs=XX[
                                            :,
                                            2 * kp : 2 * kp + 2,
                                            nt * 1024 + half * 512 : nt * 1024 + (half + 1) * 512,
                                        ],
                                        start=(kp == 0),
                                        stop=(kp == 3),
                                        perf_mode=DR,
                                    )
                            nc.vector.tensor_scalar(
                                out=outT[:, mc, nt * 1024 : (nt + 1) * 1024],
                                in0=psq,
                                scalar1=A_QK / WS,
                                scalar2=BQK[:, bcol + mc : bcol + mc + 1],
                                op0=OP.mult,
                                op1=OP.add,
                            )

            # ---------------- attention + exchange + fc ----------------
            with (
                tc.tile_pool(name="attp", bufs=1) as attp,
                tc.tile_pool(name="late", bufs=1) as late,
            ):
                GAM = late.tile([P, D], F32, tag="GAM")
                BET = late.tile([P, D], F32, tag="BET")
                nc.sync.dma_start(out=GAM, in_=gam_d.ap().to_broadcast([P, D]))
                nc.sync.dma_start(out=BET, in_=bet_d.ap().to_broadcast([P, D]))

                for qc in range(QC):
                    qlo = qc * 1024
                    pvss = []
                    for hp in range(4):
                        EXPT = attp.tile([P, KC, 2048], FP8E5, tag="exp", bufs=2)
                        pvs = [
                            ps.tile([65, 1024], F32, tag="pv", name=f"pv{hb}")
                            for hb in range(2)
                        ]
                        for kc in range(KC):
                            klo = kc * P
                            for hb in range(2):
                                plo = hb * 64
                                sc = ps.tile([P, 1024], F32, tag="sc")
                                for half in range(2):
                                    nc.tensor.matmul(
                                        sc[:, half * 512 : (half + 1) * 512],
                                        lhsT=KT[plo : plo + 64, hp, klo : klo + P],
                                        rhs=QT[
                                            plo : plo + 64,
                                            hp,
                                            qlo + half * 512 : qlo + (half + 1) * 512,
                                        ],
                                    )
                                edst = EXPT[:, kc, hb * 1024 : (hb + 1) * 1024]
                                if _vec_owned(kc, hb):
                                    nc.vector.tensor_scalar(
                                        out=edst.bitcast(U8),
                                        in0=sc,
                                        scalar1=C2,
                                        scalar2=0.0,
                                        op0=OP.add,
                                        op1=OP.max,
                                    )
                                else:
                                    nc.scalar.activation(
                                        out=edst, in_=sc, func=AF.Exp, scale=EXP_SCALE
                                    )
                            if kc % 2 == 1:
                                kp = kc // 2
                                for hb in range(2):
                                    for half in range(2):
                                        nc.tensor.matmul(
                                            pvs[hb][0:65, half * 512 : (half + 1) * 512],
                                            lhsT=VO[:, kp, :, 2 * hp + hb, 0:65],
                                            rhs=EXPT[
                                                :,
                                                2 * kp : 2 * kp + 2,
                                                hb * 1024 + half * 512 : hb * 1024 + (half + 1) * 512,
                                            ],
                                            start=(kp == 0),
                                            stop=(kp == 7),
                                            perf_mode=DR,
                                        )
                        if dbg and qc == 0 and hp == 0:
                            nc.sync.dma_start(out=dbg_exp.ap(), in_=EXPT)
                        # move pv to SBUF right away (frees PSUM for next hp);
                        # normalization is deferred until the per-qc batched
                        # reciprocal is done.
                        PVS = attp.tile([65, 2, 1024], BF16, tag="PVS", bufs=4)
                        pvss.append(PVS)
                        nc.scalar.copy(PVS[:, 0, :], pvs[0])
                        nc.vector.tensor_copy(PVS[:, 1, :], pvs[1])
                        rlo = qc * HL + 2 * hp
                        for hb in range(2):
                            nc.sync.dma_start(
                                out=sums_dram[rlo + hb : rlo + hb + 1, :],
                                in_=PVS[64:65, hb, :],
                            )

                    # batched reciprocal of all 8 heads' sums on 8 lanes
                    SUI = attp.tile([8, 1024], BF16, tag="SUI", bufs=2)
                    SUO = attp.tile([8, 1024], F32, tag="SUO", bufs=2)
                    nc.sync.dma_start(
                        out=SUI, in_=sums_dram[qc * HL : (qc + 1) * HL, :]
                    )
                    nc.vector.reciprocal(SUO, SUI)
                    nc.sync.dma_start(
                        out=rec_dram[qc * HL : (qc + 1) * HL, :], in_=SUO
                    )
                    for hp in range(4):
                        rb = attp.tile([64, 2, 1024], F32, tag="rb", bufs=2)
                        for hb in range(2):
                            nc.sync.dma_start(
                                out=rb[:, hb, :],
                                in_=rec_dram[
                                    qc * HL + 2 * hp + hb : qc * HL + 2 * hp + hb + 1, :
                                ].to_broadcast([64, 1024]),
                            )
                        # ATT (fp8, x16 context) = pv * (2/sums); pv carries x32
                        for hb in range(2):
                            nc.vector.tensor_tensor(
                                ATT[hb * 64 : (hb + 1) * 64, hp, qlo : qlo + 1024],
                                pvss[hp][0:64, hb, :],
                                rb[:, hb, :],
                                OP.mult,
                            )

                    if dbg and qc == 0:
                        nc.sync.dma_start(out=dbg_qt.ap(), in_=QT)
                        nc.sync.dma_start(out=dbg_kt.ap(), in_=KT)
                        nc.sync.dma_start(out=dbg_vo.ap(), in_=VO)
                        nc.sync.dma_start(out=dbg_att.ap(), in_=ATT)
                        nc.sync.dma_start(
                            out=dbg_rec[0:HL, :], in_=rec_dram[0:HL, :]
                        )
                    # fc partials (fp8 DR, K=512) over all 2048 tokens of this
                    # q-half, bf16 into cc_in chunks; RS chunk fires when its
                    # 4 token-blocks are done.  chunk rows:
                    #   [ch*256+off of member-0 region | same of member-1].
                    for ch in range(2):
                        for sub in range(4):
                            j, r = divmod(sub, 2)          # pair member, block
                            t = j * 4 + ch * 2 + r          # tokc within q-half
                            psf = ps.tile([P, 1024], F32, tag="sc", name="psf")
                            for dp in range(2):
                                for half in range(2):
                                    nc.tensor.matmul(
                                        psf[:, half * 512 : (half + 1) * 512],
                                        lhsT=ATT[
                                            :, 2 * dp : 2 * dp + 2, qlo + t * P : qlo + (t + 1) * P
                                        ],
                                        rhs=WOp[
                                            :, 2 * dp : 2 * dp + 2, half * 512 : (half + 1) * 512
                                        ],
                                        start=(dp == 0),
                                        stop=(dp == 1),
                                        perf_mode=DR,
                                    )
                            fcs = late.tile([P, D], BF16, tag="fcs", bufs=4)
                            if sub % 2 == 0:
                                nc.scalar.mul(fcs, psf, 1.0 / 512.0)
                            else:
                                nc.vector.tensor_scalar_mul(
                                    out=fcs, in0=psf, scalar1=1.0 / 512.0
                                )
                            nc.sync.dma_start(
                                out=cc_in[2 * qc + ch][j * 256 + r * P : j * 256 + (r + 1) * P, :],
                                in_=fcs,
                            )
                        if dbg and qc == 0 and ch == 0:
                            nc.sync.dma_start(out=dbg_fc.ap(), in_=cc_in[0].ap())
                        nc.gpsimd.collective_compute(
                            "ReduceScatter",
                            OP.add,
                            replica_groups=groups,
                            ins=[cc_in[2 * qc + ch].ap().opt()],
                            outs=[cc_out[2 * qc + ch].ap().opt()],
                        )

                    # epilogue: +residual(+bo), LayerNorm over my 512 tokens
                    MV = late.tile([P, 4, 2], F32, tag="MV", bufs=2)
                    RST = late.tile([P, 4], F32, tag="RST", bufs=2)
                    xts = []
                    for tc4 in range(4):
                        ch, r = divmod(tc4, 2)
                        xc = late.tile([P, D], BF16, tag="xc", bufs=4)
                        nc.sync.dma_start(
                            out=xc, in_=cc_out[2 * qc + ch][r * P : (r + 1) * P, :]
                        )
                        qr = late.tile([P, D], F32, tag="qr", bufs=2)
                        nc.sync.dma_start(
                            out=qr,
                            in_=qrb_d[qc * 512 + tc4 * P : qc * 512 + (tc4 + 1) * P, :],
                        )
                        xt = late.tile([P, D], F32, tag="xt", bufs=4)
                        xts.append(xt)
                        nc.vector.tensor_tensor(xt, xc, qr, OP.add)
                        st = late.tile([P, 2, 6], F32, tag="st", bufs=2)
                        nc.vector.bn_stats(st[:, 0, :], xt[:, 0:512])
                        nc.vector.bn_stats(st[:, 1, :], xt[:, 512:1024])
                        nc.vector.bn_aggr(MV[:, tc4, :], st)
                        nc.vector.tensor_scalar_add(
                            out=RST[:, tc4 : tc4 + 1],
                            in0=MV[:, tc4, 1:2],
                            scalar1=LN_EPS,
                        )
                    nc.vector.reciprocal(RST, RST)
                    nc.scalar.activation(out=RST, in_=RST, func=AF.Sqrt)
                    for tc4 in range(4):
                        xn = late.tile([P, D], F32, tag="xn", bufs=2)
                        nc.vector.tensor_scalar(
                            out=xn,
                            in0=xts[tc4],
                            scalar1=MV[:, tc4, 0:1],
                            scalar2=RST[:, tc4 : tc4 + 1],
                            op0=OP.subtract,
                            op1=OP.mult,
                        )
                        geng = nc.gpsimd if qc == 0 else nc.vector
                        geng.tensor_tensor(xn, xn, GAM, OP.mult)
                        geng.tensor_tensor(xn, xn, BET, OP.add)
                        nc.sync.dma_start(
                            out=out_d[qc * 512 + tc4 * P : qc * 512 + (tc4 + 1) * P, :],
                            in_=xn,
                        )

    nc.compile()
    return nc


def _f8(a):
    return np.ascontiguousarray(a).astype(ml_dtypes.float8_e4m3)


def kernel(q, k, v, Wq, bq, Wk, bk, Wv, bv, Wo, bo, gamma, beta, _trace=False):
    global _NC_CACHE, _LAST_RES
    q = np.asarray(q, np.float32)
    k = np.asarray(k, np.float32)
    v = np.asarray(v, np.float32)
    Wq, Wk, Wv, Wo = (np.asarray(w, np.float32) for w in (Wq, Wk, Wv, Wo))
    bq, bk, bv, bo = (np.asarray(x, np.float32) for x in (bq, bk, bv, bo))
    gamma = np.asarray(gamma, np.float32)
    beta = np.asarray(beta, np.float32)

    in_maps = []
    for c in range(8):
        b, g = divmod(c, 2)
        sl = slice(g * DH, (g + 1) * DH)
        qres = np.concatenate(
            [
                q[b, g * 512 : g * 512 + 512],
                q[b, 1024 + g * 512 : 1024 + g * 512 + 512],
            ]
        )
        in_maps.append(
            {
                "xqT": _f8(q[b].T),
                "xkT": _f8(k[b].T),
                "xvT": _f8(v[b].T),
                "wqT": _f8(Wq[sl, :].T * WS),
                "wkT": _f8(Wk[sl, :].T * WS),
                "wvT": _f8(Wv[sl, :].T * WS),
                "woT": _f8(Wo[:, sl].T * WS),
                "bq": (bq[sl] * A_QK).astype(np.float32),
                "bk": (bk[sl] * A_QK).astype(np.float32),
                "bv": (bv[sl] * WS).reshape(1, DH).astype(np.float32),
                "gamma": gamma.reshape(1, D).copy(),
                "beta": beta.reshape(1, D).copy(),
                "qrb": np.ascontiguousarray(qres + bo[None, :]),
            }
        )

    if _NC_CACHE is None:
        _NC_CACHE = build_nc()
    nc = _NC_CACHE

    kw = {}
    if _trace:
        import tempfile

        kw = dict(trace=True, tmpdir=tempfile.mkdtemp(prefix="mha_trace_"))
    res = run_bass_kernel_spmd(nc, in_maps, list(range(8)), **kw)
    _LAST_RES = res

    out = np.empty((B, S, D), np.float32)
    for c in range(8):
        b, g = divmod(c, 2)
        r = res.results[c]["out"]
        out[b, g * 512 : g * 512 + 512] = r[0:512]
        out[b, 1024 + g * 512 : 1024 + g * 512 + 512] = r[512:1024]

    if _trace:
        kernel._last = res
    return out

